# revision 23
# baseline (speedup 1.0000x reference)
"""Multi-head attention (B=2, S=2048, D=1024, H=16) on 8 TRN2 NeuronCores.

Sharding: batch x head-group. Core c handles batch c//4 and heads
4*(c%4) .. 4*(c%4)+3 (4 heads, organized as 2 pairs of 2).

Per-core dataflow (all on-chip, no device-side transposes):
  host passes x^T = q/k/v[b].T so contraction dims land on partitions.
    K^T[2*64, seq]  = Wk_g^T x^T   (fp32r matmuls, PE)
    Q^T[2*64, seq]  = Wq_g^T x^T
    V[seq, 4*65]    = x^T.T Wv     (bf16, with a constant ones column per head)
    S^T[k, q]       = K Q^T        (fp32r, 2 heads row-packed in the PE array)
    P^T             = exp(S^T)     (ScalarE, PSUM->SBUF, bf16 out)
    O^T[65, q]      = (V|1)^T P^T  (bf16; row 64 = softmax denominator)
    A^T             = O^T[0:64] / O^T[64]   (gpsimd partition-broadcast + DVE)
    out^T[e, q]    += Wo_slice A^T (fp32r, partial over this core's 256 dims)
  host sums the 4 partial out^T per batch, transposes, adds bo.
"""
import numpy as np
import ml_dtypes

import concourse.bacc as bacc
import concourse.tile as tile
from concourse import mybir
from concourse import bass_utils

S = 2048          # sequence length
D = 1024          # model dim
H = 16            # total heads
DK = 64           # head dim
NCORES = 8
HPC = 4           # heads per core
G = 2             # head pairs per core
P = 128
CH = D // P       # contraction chunks over model dim
NQT = S // 512    # 512-wide q tiles
NKT = S // P      # 128-wide k tiles


F32R = mybir.dt.float32r
F32 = mybir.dt.float32
BF16 = mybir.dt.bfloat16

_prog_cache = {}


def _build_program():
    nc = bacc.Bacc("TRN2", target_bir_lowering=False, debug=False,
                   num_devices=NCORES)

    xq_d = nc.dram_tensor("xq", [D, S], F32R, kind="ExternalInput").ap()
    xk_d = nc.dram_tensor("xk", [D, S], F32R, kind="ExternalInput").ap()
    xv_d = nc.dram_tensor("xv", [D, S], BF16, kind="ExternalInput").ap()
    wq_d = nc.dram_tensor("wq", [D, HPC * DK], F32R, kind="ExternalInput").ap()
    wk_d = nc.dram_tensor("wk", [D, HPC * DK], F32R, kind="ExternalInput").ap()
    wv_d = nc.dram_tensor("wv", [D, HPC * DK], BF16, kind="ExternalInput").ap()
    wo_d = nc.dram_tensor("wo", [HPC * DK, D], F32R, kind="ExternalInput").ap()
    bq_d = nc.dram_tensor("bq", [P, G], F32, kind="ExternalInput").ap()
    bk_d = nc.dram_tensor("bk", [P, G], F32, kind="ExternalInput").ap()
    bv_d = nc.dram_tensor("bv", [P, HPC * DK], F32, kind="ExternalInput").ap()
    out_d = nc.dram_tensor("out_t", [D, S], F32, kind="ExternalOutput").ap()

    # partition-inner views for DMA
    xq_v = xq_d.rearrange("(c p) s -> p c s", p=P)
    xk_v = xk_d.rearrange("(c p) s -> p c s", p=P)
    xv_v = xv_d.rearrange("(c p) s -> p c s", p=P)
    wq_v = wq_d.rearrange("(c p) n -> p c n", p=P)
    wk_v = wk_d.rearrange("(c p) n -> p c n", p=P)
    wv_v = wv_d.rearrange("(c p) n -> p c n", p=P)
    wo_v = wo_d.rearrange("(c p) e -> p c e", p=P)   # [128, 2, 1024]

    with tile.TileContext(nc) as tc:
        with tc.tile_pool(name="singles", bufs=1) as singles, \
             tc.tile_pool(name="xqk", bufs=2) as xqk_pool, \
             tc.tile_pool(name="xv", bufs=3) as xv_pool, \
             tc.tile_pool(name="pt", bufs=14) as pt_pool, \
             tc.tile_pool(name="at", bufs=3) as at_pool, \
             tc.tile_pool(name="rb", bufs=2) as rb_pool, \
             tc.tile_pool(name="ostage", bufs=3) as ostage_pool, \
             tc.tile_pool(name="ps_proj", bufs=2, space="PSUM") as ps_proj, \
             tc.tile_pool(name="ps_slab", bufs=2, space="PSUM") as ps_slab, \
             tc.tile_pool(name="ps_pv", bufs=2, space="PSUM") as ps_pv:

            # ---- weights / biases resident in SBUF ----
            wq_sb = singles.tile([P, CH, HPC * DK], F32R, tag="wq")
            wk_sb = singles.tile([P, CH, HPC * DK], F32R, tag="wk")
            wv_sb = singles.tile([P, CH, HPC * DK], BF16, tag="wv")
            wo_sb = singles.tile([P, G, D], F32R, tag="wo")
            bq_sb = singles.tile([P, G], F32, tag="bq")
            bk_sb = singles.tile([P, G], F32, tag="bk")
            bv_sb = singles.tile([P, HPC * DK], F32, tag="bv")
            # critical path first: the first K-projection matmuls need wk
            # chunk c + xk chunk c — spread those across queues ahead of the
            # bulk weight traffic
            for c in range(CH):
                nc.sync.dma_start(wk_sb[:, c, :], wk_v[:, c, :])
            nc.sync.dma_start(bk_sb[:], bk_d)
            nc.sync.dma_start(bq_sb[:], bq_d)

            # persistent activations
            qt_sb = singles.tile([P, G, S], F32R, tag="qt")    # Q^T
            kt_sb = singles.tile([P, G, S], F32R, tag="kt")    # K^T
            v_sb = singles.tile([P, NKT, HPC * 65], BF16, tag="v")  # V | ones

            # warm the exp table while projections run
            warm = singles.tile([P, G], F32, tag="warm")
            nc.scalar.activation(warm[:], bq_sb[:],
                                 mybir.ActivationFunctionType.Exp)
            # warm the PE clock (HAM) with a dummy matmul chain while the
            # first input DMAs are still in flight
            wtile0 = singles.tile([P, 512], F32, tag="wtile0")
            nc.vector.memset(wtile0[:], 0.0)
            wtile = singles.tile([P, 512], F32R, tag="wtile")
            nc.vector.tensor_copy(wtile[:], wtile0[:])
            wps = ps_pv.tile([P, 512], F32, tag="pv", name="warm_ps")
            for i in range(40):
                nc.tensor.matmul(wps[:], wtile[:, 0:P], wtile[:],
                                 start=True, stop=True)
            # ones columns of V (projection writes only the 64-wide blocks)
            nc.vector.memset(v_sb[:], 1.0)

            def qk_proj(ks, which):
                """Project one 1024-wide seq slice pair of q or k.

                Two 512 q-slices share each stationary weight load and land in
                one [128, 1024] PSUM slab (borrowed from the attention pool).
                """
                x_v, w_sb, b_sb, dst = (
                    (xq_v, wq_sb, bq_sb, qt_sb) if which == "q"
                    else (xk_v, wk_sb, bk_sb, kt_sb)
                )
                xt = xqk_pool.tile([P, CH, 1024], F32R, tag="xqk")
                # per-chunk DMAs so the first accumulation matmul starts as
                # soon as chunk 0 lands instead of waiting for the full slice;
                # halves spread single chunks across two queues
                for c in range(CH):
                    for j in range(2):
                        nc.sync.dma_start(
                            xt[:, c, j * 512:(j + 1) * 512],
                            x_v[:, c, ks * 1024 + j * 512:
                                ks * 1024 + (j + 1) * 512])
                for g in range(G):
                    pt = ps_slab.tile([P, 1024], F32, tag="slab",
                                      name=f"proj_{which}_{ks}_{g}")
                    for c in range(CH):
                        w_ap = (wq_sb if which == "q" else wk_sb)[
                            :, c, g * P:(g + 1) * P]
                        for j in range(2):
                            nc.tensor.matmul(
                                pt[:, j * 512:(j + 1) * 512], w_ap,
                                xt[:, c, j * 512:(j + 1) * 512],
                                start=(c == 0), stop=(c == CH - 1))
                    nc.vector.tensor_tensor(
                        dst[:, g, ks * 1024:(ks + 1) * 1024], pt[:],
                        b_sb[:, g:g + 1].to_broadcast([P, 1024]),
                        mybir.AluOpType.add)

            def v_proj(ct):
                """Project one 128-wide seq slice of v into V[seq, 4*65]."""
                xt = xv_pool.tile([P, CH, P], BF16, tag="xv")
                nc.sync.dma_start(xt[:], xv_v[:, :, ct * P:(ct + 1) * P])
                pt = ps_proj.tile([P, 512], F32, tag="proj")
                pv = pt[:, :HPC * DK]
                for c in range(CH):
                    nc.tensor.matmul(pv, xt[:, c, :], wv_sb[:, c, :],
                                     start=(c == 0), stop=(c == CH - 1))
                dst = v_sb[:, ct, :].rearrange("p (h x) -> p h x", h=HPC)[:, :, 0:DK]
                nc.vector.tensor_tensor(
                    dst, pv.rearrange("p (h x) -> p h x", h=HPC),
                    bv_sb[:].rearrange("p (h x) -> p h x", h=HPC),
                    mybir.AluOpType.add)

            # ---- K projection first (attention needs all of K^T) ----
            qk_proj(0, "k")
            # bulk weight traffic after the critical first slice is in flight
            for c in range(CH):
                nc.sync.dma_start(wq_sb[:, c, :], wq_v[:, c, :])
            nc.gpsimd.dma_start(wv_sb[:], wv_v)
            nc.gpsimd.dma_start(bv_sb[:], bv_d)
            for c in range(G):
                nc.sync.dma_start(wo_sb[:, c, :], wo_v[:, c, :])
            qk_proj(1, "k")
            qk_proj(0, "q")

            for qt in range(NQT):
                at = at_pool.tile([P, G, 512], F32R, tag="at")
                for g in range(G):
                    pv_ps = [ps_pv.tile([65, 512], F32, tag="pv",
                                        name=f"pv_{qt}_{g}_{i}")
                             for i in range(2)]
                    for kt in range(NKT):
                        if qt == 0 and g == 0:
                            v_proj(kt)
                        # one slab holds both heads of the pair for this k tile
                        slab = ps_slab.tile([P, 1024], F32, tag="slab",
                                            name=f"slab_{qt}_{g}_{kt}")
                        for hh in range(2):   # row-packed pair
                            lo = hh * DK
                            nc.tensor.matmul(
                                slab[:, hh * 512:(hh + 1) * 512],
                                kt_sb[lo:lo + DK, g, kt * P:(kt + 1) * P],
                                qt_sb[lo:lo + DK, g, qt * 512:(qt + 1) * 512],
                                start=True, stop=True)
                        ptile = pt_pool.tile([P, 2, 512], BF16, tag="pt")
                        nc.scalar.activation(
                            ptile[:], slab[:],
                            mybir.ActivationFunctionType.Exp)
                        for hh in range(2):
                            h = 2 * g + hh
                            nc.tensor.matmul(
                                pv_ps[hh],
                                v_sb[:, kt, h * 65:(h + 1) * 65],
                                ptile[:, hh, :],
                                start=(kt == 0), stop=(kt == NKT - 1))
                    for hh in range(2):
                        h = 2 * g + hh
                        # denominator lives in psum row 64; reshape to [64, 8]
                        # across partitions for a cheap reciprocal, then
                        # broadcast across partitions and multiply
                        srow = rb_pool.tile([P, 512], F32, tag="srow")
                        nc.vector.tensor_copy(srow[64:65, :], pv_ps[hh][64:65, :])
                        rs = rb_pool.tile([DK, 8], F32, tag="rs")
                        nc.sync.dma_start(rs[:], srow[64:65, :])
                        nc.vector.reciprocal(rs[:], rs[:])
                        srow0 = rb_pool.tile([1, 512], F32, tag="srow0")
                        nc.sync.dma_start(srow0[0:1, :], rs[:])
                        rb = rb_pool.tile([DK, 512], F32, tag="rb")
                        nc.gpsimd.partition_broadcast(rb[:], srow0[0:1, :])
                        if hh == 0:
                            nc.vector.tensor_tensor(
                                at[0:DK, g, :], pv_ps[hh][0:DK, :], rb[:],
                                mybir.AluOpType.mult)
                        else:
                            od = rb_pool.tile([DK, 512], F32R, tag="od")
                            nc.vector.tensor_tensor(
                                od[:], pv_ps[hh][0:DK, :], rb[:],
                                mybir.AluOpType.mult)
                            nc.gpsimd.tensor_copy(at[DK:P, g, :], od[:])

                if qt == 0:
                    qk_proj(1, "q")

                # ---- output projection for this q tile ----
                for et in range(CH):
                    po = ps_proj.tile([P, 512], F32, tag="proj")
                    for c in range(G):
                        nc.tensor.matmul(
                            po[:], wo_sb[:, c, et * P:(et + 1) * P],
                            at[:, c, :],
                            start=(c == 0), stop=(c == G - 1))
                    ost = ostage_pool.tile([P, 512], F32, tag="ost")
                    nc.vector.tensor_copy(ost[:], po[:])
                    nc.sync.dma_start(
                        out_d[et * P:(et + 1) * P, qt * 512:(qt + 1) * 512],
                        ost[:])

    nc.compile()
    return nc


def _prep_inputs(q, k, v, Wq, bq, Wk, bk, Wv, bv, Wo, bo):
    """Build the per-core input maps (host-side shard + layout prep)."""
    bf16 = ml_dtypes.bfloat16
    xt = {}
    for b in range(2):
        xt[("q", b)] = np.ascontiguousarray(q[b].T, dtype=np.float32)
        xt[("k", b)] = np.ascontiguousarray(k[b].T, dtype=np.float32)
        xt[("v", b)] = np.ascontiguousarray(v[b].T).astype(bf16)

    in_maps = []
    for core in range(NCORES):
        b = core // 4
        h0 = (core % 4) * HPC
        sl = slice(h0 * DK, (h0 + HPC) * DK)
        wq_t = np.ascontiguousarray(Wq[sl, :].T, dtype=np.float32)
        wk_t = np.ascontiguousarray(Wk[sl, :].T, dtype=np.float32)
        wv_t = np.ascontiguousarray(Wv[sl, :].T).astype(bf16)
        wo_t = np.ascontiguousarray(Wo[:, sl].T, dtype=np.float32)
        bq_a = np.ascontiguousarray(
            np.asarray(bq[sl], dtype=np.float32).reshape(G, P).T)
        bk_a = np.ascontiguousarray(
            np.asarray(bk[sl], dtype=np.float32).reshape(G, P).T)
        bv_a = np.ascontiguousarray(
            np.tile(np.asarray(bv[sl], dtype=np.float32)[None, :], (P, 1)))
        in_maps.append({
            "xq": xt[("q", b)], "xk": xt[("k", b)], "xv": xt[("v", b)],
            "wq": wq_t, "wk": wk_t, "wv": wv_t, "wo": wo_t,
            "bq": bq_a, "bk": bk_a, "bv": bv_a,
        })
    return in_maps


def kernel(q, k, v, Wq, bq, Wk, bk, Wv, bv, Wo, bo, _trace=False):
    q = np.asarray(q, dtype=np.float32)
    k = np.asarray(k, dtype=np.float32)
    v = np.asarray(v, dtype=np.float32)
    Wq = np.asarray(Wq, dtype=np.float32)
    Wk = np.asarray(Wk, dtype=np.float32)
    Wv = np.asarray(Wv, dtype=np.float32)
    Wo = np.asarray(Wo, dtype=np.float32)

    if "nc" not in _prog_cache:
        _prog_cache["nc"] = _build_program()
    nc = _prog_cache["nc"]

    in_maps = _prep_inputs(q, k, v, Wq, bq, Wk, bk, Wv, bv, Wo, bo)
    kwargs = {}
    if _trace:
        kwargs = dict(trace=True, trace_cores=[0])
    res = bass_utils.run_bass_kernel_spmd(
        nc, in_maps, core_ids=list(range(NCORES)), **kwargs)

    out = np.empty((2, S, D), dtype=np.float32)
    bo32 = np.asarray(bo, dtype=np.float32)
    for b in range(2):
        acc = res.results[4 * b]["out_t"].astype(np.float32)
        for c in range(4 * b + 1, 4 * b + 4):
            acc = acc + res.results[c]["out_t"]
        out[b] = acc.T + bo32[None, :]
    if _trace:
        return out, res
    return out
